# revision 1
# baseline (speedup 1.0000x reference)
"""DLRM DotInteraction kernel for 8x Trainium2 NeuronCores.

Full input x: [16384, 32, 64] f32. Per batch b: G = x_b @ x_b^T [32, 32];
output = strict lower triangle of G, row-major -> [16384, 496] f32.

Sharding: pure data parallel, 2048 batches per core.

Host-side prep (part of sharding/marshalling): x is retiled to
  xp[t, d, j*32 + f] = x[4*t + j, f, d]
i.e. [B/4, 64, 128] tiles holding 4 batches of x^T each, d on partitions.

Per-core dataflow:
  - DMA loads of xp tiles (contiguous 512B rows), 4 tiles per DMA.
  - per batch: one PE matmul G_b = T_b^T @ T_b with K=64, M=N=32 at
    subarray col-group position (0, 32*j) -> compact PSUM [32, 32]
    blocks; 16 slots x 4 col-groups = 64 batches per PSUM bank.
  - per bank: one DVE copy PSUM -> SBUF staging S.
  - per 512 batches: 31 strided DMAs gather the triangle rows
    (batch-major on the free dim) straight to DRAM out.
"""

import numpy as np

import concourse.bass as bass
import concourse.tile as tile
from concourse import mybir
from concourse.tile import add_dep_helper
from concourse.bass_utils import run_bass_kernel_spmd

N_CORES = 8
B_FULL = 16384
B = B_FULL // N_CORES  # 2048 batches per core
F = 32
D = 64
NPAIR = F * (F - 1) // 2  # 496

# compute dtype for the matmul operands: float32 (exact, ~4x slower PE)
# or float16 (rel err ~1e-3, 1 cyc/row).
COMPUTE_DT = mybir.dt.float32
COMPUTE_NP = np.float32

FP32 = mybir.dt.float32

TILE4 = 4            # batches per xp tile
LOAD_TILES = 8       # xp tiles per DMA load (32 batches)
BANK = 64            # batches per PSUM gram bank
GROUP = 256          # batches per staging/gather group


def split_multiwait_insts(nc):
    """walrus in this env allows only one sem wait per instruction; the tile
    tail drain carries several. Hoist extras onto preceding single-wait NoOps."""
    for func in nc.m.functions:
        for blk in func.blocks:
            insts = list(blk.instructions)
            changed = False
            new_list = []
            for inst in insts:
                si = inst.sync_info
                if si is not None and len(si.on_wait) > 1:
                    waits = list(si.on_wait)
                    for k, w in enumerate(waits[1:]):
                        new_list.append(
                            mybir.InstNoOp(
                                name=f"{inst.name}-wsplit{k}",
                                engine=inst.engine,
                                sync_info=mybir.SyncInfo(on_wait=[w], on_update=[]),
                                bass_nofuse=True,
                            )
                        )
                    inst.sync_info = mybir.SyncInfo(
                        on_wait=[waits[0]], on_update=list(si.on_update)
                    )
                    changed = True
                new_list.append(inst)
            if changed:
                blk.instructions = new_list


def host_prep(x):
    """[B, 32, 64] -> [B/4, 64, 128] per-batch transposed tiles."""
    b = x.shape[0]
    t = x.reshape(b // TILE4, TILE4, F, D).transpose(0, 3, 1, 2)  # [t, d, j, f]
    return np.ascontiguousarray(
        t.reshape(b // TILE4, D, TILE4 * F).astype(COMPUTE_NP)
    )


def build_program():
    nc = bass.Bass()
    xp = nc.declare_dram_parameter("xp", [B // TILE4, D, TILE4 * F], COMPUTE_DT,
                                   isOutput=False)
    # raw staging dump: dump[grp, 32j+f, 32s+g] = G[grp*GROUP+4s+j][f, g];
    # the triangle selection/reorder happens host-side during unshard.
    dump = nc.declare_dram_parameter(
        "dump", [B // GROUP, 128, (GROUP // 4) * F], FP32, isOutput=True
    )

    n_groups = B // GROUP
    banks_per_group = GROUP // BANK
    tiles_per_bank = BANK // TILE4          # 16 xp tiles per bank
    loads_per_bank = tiles_per_bank // LOAD_TILES

    with tile.TileContext(nc) as tc:
        with (
            tc.tile_pool(name="xin", bufs=4) as xpool,
            tc.tile_pool(name="stage", bufs=3) as spool,
            tc.tile_pool(name="psum_g", bufs=4, space="PSUM") as psumG,
        ):
            for grp in range(n_groups):
                S = spool.tile([128, (GROUP // 4) * F], FP32)  # [128, 4096]
                s_copies = []
                for bk in range(banks_per_group):
                    bank_b0 = grp * GROUP + bk * BANK
                    pG = psumG.tile([128, (BANK // 4) * F], FP32)  # [128, 512]
                    for ld in range(loads_per_bank):
                        t0 = bank_b0 // TILE4 + ld * LOAD_TILES
                        X = xpool.tile([D, LOAD_TILES * TILE4 * F], COMPUTE_DT)
                        nc.sync.dma_start(
                            X[:], xp[t0 : t0 + LOAD_TILES].transpose([1, 0, 2])
                        )
                        for tt in range(LOAD_TILES):
                            T = X[:, tt * 128 : (tt + 1) * 128]
                            for j in range(TILE4):
                                bb = (ld * LOAD_TILES + tt) * TILE4 + j
                                s, jc = bb // 4, bb % 4
                                op = T[:, j * F : (j + 1) * F]
                                nc.tensor.matmul(
                                    pG[
                                        32 * jc : 32 * jc + 32,
                                        F * s : F * s + F,
                                    ],
                                    lhsT=op,
                                    rhs=op,
                                    start=True,
                                    stop=True,
                                    tile_position=(0, 32 * jc),
                                )
                    cp = nc.vector.tensor_copy(
                        S[:, bk * (BANK // 4) * F : (bk + 1) * (BANK // 4) * F],
                        pG[:],
                    )
                    s_copies.append(cp.ins)
                # one contiguous 1MB dump per group; triangle pack on host
                g = nc.scalar.dma_start(dump[grp], S[:])
                for cp_inst in s_copies:
                    add_dep_helper(g.ins, cp_inst, sync=True)

    split_multiwait_insts(nc)
    return nc


_CACHED = None


def _get_program():
    global _CACHED
    if _CACHED is None:
        _CACHED = build_program()
    return _CACHED


_TRIL_ROWS, _TRIL_COLS = np.tril_indices(F, k=-1)


def _unpack_dump(d):
    """[B/GROUP, 128, GROUP*8] dump -> [B, 496] packed triangle rows."""
    g = d.reshape(B // GROUP, 4, F, GROUP // 4, F)      # [grp, j, f, s, g]
    g = g.transpose(0, 3, 1, 2, 4).reshape(B, F, F)     # [b, f, g]
    return g[:, _TRIL_ROWS, _TRIL_COLS]


def kernel(**inputs) -> np.ndarray:
    x = np.asarray(inputs["x"], dtype=np.float32)
    assert x.shape == (B_FULL, F, D), x.shape
    nc = _get_program()
    in_maps = [host_prep(x[i * B : (i + 1) * B]) for i in range(N_CORES)]
    res = run_bass_kernel_spmd(
        nc, [{"xp": m} for m in in_maps], list(range(N_CORES))
    )
    return np.concatenate(
        [_unpack_dump(res.results[i]["dump"]) for i in range(N_CORES)], axis=0
    ).astype(np.float32)



# revision 3
# speedup vs baseline: 1.7747x; 1.7747x over previous
"""DLRM DotInteraction kernel for 8x Trainium2 NeuronCores.

Full input x: [16384, 32, 64] f32. Per batch b: G = x_b @ x_b^T [32, 32];
output = strict lower triangle of G, row-major -> [16384, 496] f32.

Sharding: pure data parallel, 2048 batches per core.

v3 design (quad-LD fp16, two uniform row phases):
  - inputs cast to fp16 on host and packed so every load DMA is
    [128 partitions x 4KB/partition] contiguous runs (full SDMA fan-out).
    Top 64 partitions hold one 64-batch half of a 128-batch load (d on
    partitions), bottom 64 partitions the other half. All 16 load tiles
    stay resident in SBUF (64KB/partition).
  - per 4 batches (a "quad"): ONE fp16 matmul with lhsT = rhs =
    [x_a^T|x_b^T|x_c^T|x_d^T] [64, 128] -> PSUM [128, 128]; the 4 grams
    sit on the diagonal 32x32 blocks. One LDWEIGHTS serves 4 batches and
    FWL kicks in (128-col fp16 weights).
  - HW erratum dodge: interleaving matmuls whose tile_position rows
    differ (0 vs 64) hangs the exec unit (NRT status 101; bisected on
    HW). So phase 1 runs ALL top-half quads (row 0), then a drain (the
    first phase-2 matmul explicitly waits on the last phase-1 PSUM
    copies), then phase 2 runs all bottom-half quads (row 64).
  - extraction: per 2-bank PSUM tile (8 quads = 32 batches), 4 strided
    copies (one per 32-row block j, gathering the j-th diagonal block of
    every quad slot) convert f32 -> fp16 into staging. j=0,1 on the
    Vector engine, j=2,3 on the Scalar engine.
  - dump: per 256 batches one [128, 4KB] fp16 DMA of the full grams
    (1024 vals/batch); the strict-lower-triangle gather happens on host.
"""

import numpy as np

import concourse.bass as bass
import concourse.tile as tile
from concourse import mybir
from concourse.tile import add_dep_helper
from concourse.bass_utils import run_bass_kernel_spmd

N_CORES = 8
B_FULL = 16384
B = B_FULL // N_CORES  # 2048 batches per core
F = 32
D = 64
NPAIR = F * (F - 1) // 2  # 496

FP16 = mybir.dt.float16
FP32 = mybir.dt.float32

LOAD_B = 128            # batches per input DMA ([128, 2048] fp16 tile)
N_LOADS = B // LOAD_B   # 16
PG_Q = 8                # quads per PSUM tile (2 banks, 32 batches)
QUADS_PER_PHASE = B // 2 // 4        # 256
PG_PER_PHASE = QUADS_PER_PHASE // PG_Q   # 32
GROUP_B = 256           # batches per staging/dump group
PG_PER_GROUP = 8        # PSUM tiles per group
GROUPS_PER_PHASE = 4
N_GROUPS = 8


def split_multiwait_insts(nc):
    """walrus in this env allows only one sem wait per instruction; the tile
    tail drain carries several. Hoist extras onto preceding single-wait NoOps."""
    for func in nc.m.functions:
        for blk in func.blocks:
            insts = list(blk.instructions)
            changed = False
            new_list = []
            for inst in insts:
                si = inst.sync_info
                if si is not None and len(si.on_wait) > 1:
                    waits = list(si.on_wait)
                    for k, w in enumerate(waits[1:]):
                        new_list.append(
                            mybir.InstNoOp(
                                name=f"{inst.name}-wsplit{k}",
                                engine=inst.engine,
                                sync_info=mybir.SyncInfo(on_wait=[w], on_update=[]),
                                bass_nofuse=True,
                            )
                        )
                    inst.sync_info = mybir.SyncInfo(
                        on_wait=[waits[0]], on_update=list(si.on_update)
                    )
                    changed = True
                new_list.append(inst)
            if changed:
                blk.instructions = new_list


def host_prep(xc):
    """[2048, 32, 64] f32 -> [128, 16, 2048] fp16 load tiles.

    batch b = 128*t + 64*par + 4*k + j lives at partitions 64*par + d,
    cols (load t) k*128 + j*32 + f."""
    xv = xc.reshape(N_LOADS, 2, 16, 4, F, D)   # [t, par, k, j, f, d]
    xq = xv.transpose(1, 5, 0, 2, 3, 4)        # [par, d, t, k, j, f]
    return np.ascontiguousarray(
        xq.reshape(128, N_LOADS, 2048).astype(np.float16)
    )


def build_program():
    nc = bass.Bass()
    xq = nc.declare_dram_parameter("xq", [128, N_LOADS, 2048], FP16,
                                   isOutput=False)
    dump = nc.declare_dram_parameter(
        "dump", [N_GROUPS, 128, GROUP_B * 8], FP16, isOutput=True
    )

    with tile.TileContext(nc) as tc:
        with (
            tc.tile_pool(name="xin", bufs=N_LOADS) as xpool,
            tc.tile_pool(name="stage", bufs=2) as spool,
            tc.tile_pool(name="psum_g", bufs=4, space="PSUM") as psumG,
        ):
            # all 16 loads up front; tiles stay resident for both phases
            Xs = []
            for t in range(N_LOADS):
                X = xpool.tile([128, 16, 4, F], FP16)
                nc.sync.dma_start(X[:], xq[:, t, :])
                Xs.append(X)

            prev_phase_last_copies = None
            for phase in range(2):
                for grp4 in range(GROUPS_PER_PHASE):
                    grp = phase * GROUPS_PER_PHASE + grp4
                    S = spool.tile([128, PG_PER_GROUP, PG_Q, F], FP16)
                    s_copies = []
                    for g2 in range(PG_PER_GROUP):
                        pG = psumG.tile([128, PG_Q, 4, F], FP32)  # 2 banks
                        pg_copies = []
                        for s in range(PG_Q):
                            q = (grp4 * PG_PER_GROUP + g2) * PG_Q + s
                            t, k = q // 16, q % 16
                            op = Xs[t][64 * phase : 64 * phase + 64, k]
                            mm = nc.tensor.matmul(
                                pG[:, s], lhsT=op, rhs=op, start=True, stop=True
                            )
                            if prev_phase_last_copies is not None:
                                # drain: no row-0/row-64 overlap on the PE
                                for cp_inst in prev_phase_last_copies:
                                    add_dep_helper(mm.ins, cp_inst, sync=True)
                                prev_phase_last_copies = None
                        for j in range(4):
                            src = pG[32 * j : 32 * j + 32, :, j, :]
                            dst = S[32 * j : 32 * j + 32, g2, :, :]
                            if j < 2:
                                cp = nc.vector.tensor_copy(dst, src)
                            else:
                                cp = nc.scalar.copy(dst, src)
                            s_copies.append(cp.ins)
                            pg_copies.append(cp.ins)
                    g = nc.scalar.dma_start(dump[grp], S[:])
                    for cp_inst in s_copies:
                        add_dep_helper(g.ins, cp_inst, sync=True)
                prev_phase_last_copies = pg_copies

    split_multiwait_insts(nc)
    return nc


_CACHED = None


def _get_program():
    global _CACHED
    if _CACHED is None:
        _CACHED = build_program()
    return _CACHED


_TRIL_ROWS, _TRIL_COLS = np.tril_indices(F, k=-1)


def _batch_map():
    """batch index for each (grp, j, g2, s) dump coordinate."""
    grp, j, g2, s = np.meshgrid(
        np.arange(N_GROUPS), np.arange(4), np.arange(PG_PER_GROUP),
        np.arange(PG_Q), indexing="ij",
    )
    phase = grp // GROUPS_PER_PHASE
    q = ((grp % GROUPS_PER_PHASE) * PG_PER_GROUP + g2) * PG_Q + s
    t, k = q // 16, q % 16
    return (LOAD_B * t + 64 * phase + 4 * k + j).ravel()


_BATCH_MAP = _batch_map()


def _unpack_dump(d):
    """[8, 128, 2048] fp16 dump -> [2048, 496] f32 triangle rows."""
    d6 = d.reshape(N_GROUPS, 4, F, PG_PER_GROUP, PG_Q, F)  # grp,j,f,g2,s,c
    d6 = d6.transpose(0, 1, 3, 4, 2, 5)                    # grp,j,g2,s,f,c
    G = np.empty((B, F, F), dtype=np.float32)
    G[_BATCH_MAP] = d6.reshape(-1, F, F).astype(np.float32)
    return G[:, _TRIL_ROWS, _TRIL_COLS]


def kernel(**inputs) -> np.ndarray:
    x = np.asarray(inputs["x"], dtype=np.float32)
    assert x.shape == (B_FULL, F, D), x.shape
    nc = _get_program()
    in_maps = [host_prep(x[i * B : (i + 1) * B]) for i in range(N_CORES)]
    res = run_bass_kernel_spmd(
        nc, [{"xq": m} for m in in_maps], list(range(N_CORES))
    )
    return np.concatenate(
        [_unpack_dump(res.results[i]["dump"]) for i in range(N_CORES)], axis=0
    ).astype(np.float32)


# revision 9
# speedup vs baseline: 1.8262x; 1.0290x over previous
"""DLRM DotInteraction kernel for 8x Trainium2 NeuronCores.

Full input x: [16384, 32, 64] f32. Per batch b: G = x_b @ x_b^T [32, 32];
output = strict lower triangle of G, row-major -> [16384, 496] f32.

Sharding: pure data parallel, 2048 batches per core.

v5 design (K=128 zero-padded cells, rotating PSUM column blocks):
  - X buffers [128, 2, 64, 32] fp16: region 0 holds 64 "A" batches' x^T
    on partitions 0:64 (d on partitions) with ZEROS on 64:128; region 1
    holds 64 "B" batches on 64:128 with zeros on 0:64. Zero cells are
    memset once on NB persistent buffers (~0.6us each); per-load DMAs
    (two complementary 64-partition transfers hitting disjoint SDMA
    engine sets -> full aggregate bandwidth) overwrite only data cells.
  - per batch: ONE matmul, lhsT = rhs = its cell [128, 32] (contiguous,
    single free dim). K=128 with the zero half contributing nothing, so
    every matmul runs at tile_position row 0 — dodges the HW erratum
    where mixing row offsets 0/64 hangs the exec unit. Out [32, 32]
    goes to PSUM partition block 32*j with j rotating 0..3, so four
    batches tile a fully-useful [128, 32] slot and consecutive
    LDWEIGHTS/MATMUL hit disjoint PE column groups (they overlap).
  - extraction: per PSUM bank (16 slots = 64 batches) two full-width
    [128, 8, 32] f32->fp16 copies (Vector + Scalar engines), zero
    garbage, ~2x faster than the PE stream -> no backpressure, PE duty
    stays high and the HAM clock ramps to 2.4 GHz.
  - dump: per 256 batches one [128, 4KB] fp16 DMA of the full grams;
    the strict-lower-triangle gather happens on host.
"""

import numpy as np

import concourse.bass as bass
import concourse.tile as tile
from concourse import mybir
from concourse.tile import add_dep_helper
from concourse.bass_utils import run_bass_kernel_spmd

N_CORES = 8
B_FULL = 16384
B = B_FULL // N_CORES  # 2048 batches per core
F = 32
D = 64
NPAIR = F * (F - 1) // 2  # 496

FP16 = mybir.dt.float16
FP32 = mybir.dt.float32

LOAD_B = 128            # batches per load (two half-width DMAs)
N_LOADS = B // LOAD_B   # 16
NB = 3                  # persistent X buffers
BANK_B = 64             # batches per PSUM bank (16 slots x 4 blocks / 1)
GROUP_B = 256           # batches per staging/dump group
BANKS_PER_GROUP = 4
N_BANKS = B // BANK_B   # 32
N_GROUPS = 8


def split_multiwait_insts(nc):
    """walrus in this env allows only one sem wait per instruction; the tile
    tail drain carries several. Hoist extras onto preceding single-wait NoOps."""
    for func in nc.m.functions:
        for blk in func.blocks:
            insts = list(blk.instructions)
            changed = False
            new_list = []
            for inst in insts:
                si = inst.sync_info
                if si is not None and len(si.on_wait) > 1:
                    waits = list(si.on_wait)
                    for k, w in enumerate(waits[1:]):
                        new_list.append(
                            mybir.InstNoOp(
                                name=f"{inst.name}-wsplit{k}",
                                engine=inst.engine,
                                sync_info=mybir.SyncInfo(on_wait=[w], on_update=[]),
                                bass_nofuse=True,
                            )
                        )
                    inst.sync_info = mybir.SyncInfo(
                        on_wait=[waits[0]], on_update=list(si.on_update)
                    )
                    changed = True
                new_list.append(inst)
            if changed:
                blk.instructions = new_list


def host_prep(xc):
    """[2048, 32, 64] f32 -> [128, 16, 2048] fp16.

    Row 64*par + d, col (t, p, f): batch 128*t + 64*par + p."""
    xv = xc.reshape(N_LOADS, 2, 64, F, D)      # [t, par, p, f, d]
    xq = xv.transpose(1, 4, 0, 2, 3)           # [par, d, t, p, f]
    return np.ascontiguousarray(
        xq.reshape(128, N_LOADS, 2048).astype(np.float16)
    )


def build_program():
    nc = bass.Bass()
    xq = nc.declare_dram_parameter("xq", [128, N_LOADS, 2048], FP16,
                                   isOutput=False)
    dump = nc.declare_dram_parameter(
        "dump", [N_GROUPS, 128, GROUP_B * 8], FP16, isOutput=True
    )

    with tile.TileContext(nc) as tc:
        with (
            tc.tile_pool(name="xin", bufs=NB) as xpool,
            tc.tile_pool(name="stage", bufs=2) as spool,
            tc.tile_pool(name="psum_g", bufs=6, space="PSUM") as psumG,
        ):
            # persistent X buffers; zero cells written once
            Xs = []
            for b in range(NB):
                X = xpool.tile([128, 2, 64, F], FP16)
                nc.vector.memset(X[64:128, 0], 0.0)
                nc.vector.memset(X[0:64, 1], 0.0)
                Xs.append(X)

            def load(t):
                X = Xs[t % NB]
                nc.sync.dma_start(X[0:64, 0], xq[0:64, t, :])
                nc.sync.dma_start(X[64:128, 1], xq[64:128, t, :])

            for t0 in range(NB):
                load(t0)
            for grp in range(N_GROUPS):
                S = spool.tile([128, BANKS_PER_GROUP, 16, F], FP16)
                s_copies = []
                for g4 in range(BANKS_PER_GROUP):
                    bank = grp * BANKS_PER_GROUP + g4
                    t, p0 = bank // 2, (bank % 2) * 32
                    X = Xs[t % NB]
                    pG = psumG.tile([128, 16, F], FP32)  # 1 bank, 16 slots
                    for i in range(BANK_B):
                        sl, j = i // 4, i % 4
                        r, p = i % 2, p0 + i // 2
                        cell = X[:, r, p, :]             # [128, 32]
                        nc.tensor.matmul(
                            pG[32 * j : 32 * j + 32, sl], lhsT=cell, rhs=cell,
                            start=True, stop=True, tile_position=(0, 32 * j),
                        )
                    # prefetch into the buffer this load just freed; issued
                    # after the load's last matmul so the WAR dep orders the
                    # DMA behind those reads, not ahead of them
                    if bank % 2 == 1 and t + NB < N_LOADS:
                        load(t + NB)
                    for half in range(2):
                        src = pG[:, 8 * half : 8 * half + 8, :]
                        dst = S[:, g4, 8 * half : 8 * half + 8, :]
                        if half == 0:
                            cp = nc.vector.tensor_copy(dst, src)
                        else:
                            cp = nc.scalar.copy(dst, src)
                        s_copies.append(cp.ins)
                g = nc.scalar.dma_start(dump[grp], S[:])
                for cp_inst in s_copies:
                    add_dep_helper(g.ins, cp_inst, sync=True)

    split_multiwait_insts(nc)
    return nc


_CACHED = None


def _get_program():
    global _CACHED
    if _CACHED is None:
        _CACHED = build_program()
    return _CACHED


_TRIL_ROWS, _TRIL_COLS = np.tril_indices(F, k=-1)


def _batch_map():
    """batch index for each (grp, j, g4, sl) dump coordinate.

    dump[grp, 128p, 2048]: partition p = 32*j + f; cols (g4, sl, c).
    Bank cell i = 4*sl + j held batch 128*t + 64*(i%2) + p0 + i//2."""
    grp, j, g4, sl = np.meshgrid(
        np.arange(N_GROUPS), np.arange(4), np.arange(BANKS_PER_GROUP),
        np.arange(16), indexing="ij",
    )
    bank = grp * BANKS_PER_GROUP + g4
    t, p0 = bank // 2, (bank % 2) * 32
    i = 4 * sl + j
    return (LOAD_B * t + 64 * (i % 2) + p0 + i // 2).ravel()


_BATCH_MAP = _batch_map()


def _unpack_dump(d):
    """[8, 128, 2048] fp16 dump -> [2048, 496] f32 triangle rows."""
    d6 = d.reshape(N_GROUPS, 4, F, BANKS_PER_GROUP, 16, F)  # grp,j,f,g4,sl,c
    d6 = d6.transpose(0, 1, 3, 4, 2, 5)                     # grp,j,g4,sl,f,c
    G = np.empty((B, F, F), dtype=np.float32)
    G[_BATCH_MAP] = d6.reshape(-1, F, F).astype(np.float32)
    return G[:, _TRIL_ROWS, _TRIL_COLS]


def kernel(**inputs) -> np.ndarray:
    x = np.asarray(inputs["x"], dtype=np.float32)
    assert x.shape == (B_FULL, F, D), x.shape
    nc = _get_program()
    in_maps = [host_prep(x[i * B : (i + 1) * B]) for i in range(N_CORES)]
    res = run_bass_kernel_spmd(
        nc, [{"xq": m} for m in in_maps], list(range(N_CORES))
    )
    return np.concatenate(
        [_unpack_dump(res.results[i]["dump"]) for i in range(N_CORES)], axis=0
    ).astype(np.float32)


# revision 12
# speedup vs baseline: 1.8428x; 1.0091x over previous
"""DLRM DotInteraction kernel for 8x Trainium2 NeuronCores.

Full input x: [16384, 32, 64] f32. Per batch b: G = x_b @ x_b^T [32, 32];
output = strict lower triangle of G, row-major -> [16384, 496] f32.

Sharding: pure data parallel, 2048 batches per core.

v5 design (K=128 zero-padded cells, rotating PSUM column blocks):
  - X buffers [128, 2, 64, 32] fp16: region 0 holds 64 "A" batches' x^T
    on partitions 0:64 (d on partitions) with ZEROS on 64:128; region 1
    holds 64 "B" batches on 64:128 with zeros on 0:64. Zero cells are
    memset once on NB persistent buffers (~0.6us each); per-load DMAs
    (two complementary 64-partition transfers hitting disjoint SDMA
    engine sets -> full aggregate bandwidth) overwrite only data cells.
  - per batch: ONE matmul, lhsT = rhs = its cell [128, 32] (contiguous,
    single free dim). K=128 with the zero half contributing nothing, so
    every matmul runs at tile_position row 0 — dodges the HW erratum
    where mixing row offsets 0/64 hangs the exec unit. Out [32, 32]
    goes to PSUM partition block 32*j with j rotating 0..3, so four
    batches tile a fully-useful [128, 32] slot and consecutive
    LDWEIGHTS/MATMUL hit disjoint PE column groups (they overlap).
  - extraction: per PSUM bank (16 slots = 64 batches) two full-width
    [128, 8, 32] f32->fp16 copies (Vector + Scalar engines), zero
    garbage, ~2x faster than the PE stream -> no backpressure, PE duty
    stays high and the HAM clock ramps to 2.4 GHz.
  - dump: per 256 batches one [128, 4KB] fp16 DMA of the full grams;
    the strict-lower-triangle gather happens on host.
"""

import numpy as np

import concourse.bass as bass
import concourse.tile as tile
from concourse import mybir
from concourse.tile import add_dep_helper
from concourse.bass_utils import run_bass_kernel_spmd

N_CORES = 8
B_FULL = 16384
B = B_FULL // N_CORES  # 2048 batches per core
F = 32
D = 64
NPAIR = F * (F - 1) // 2  # 496

FP16 = mybir.dt.float16
FP32 = mybir.dt.float32

LOAD_B = 128            # batches per load (two half-width DMAs)
N_LOADS = B // LOAD_B   # 16
NB = 3                  # persistent X buffers
BANK_B = 64             # batches per PSUM bank (16 slots x 4 blocks / 1)
GROUP_B = 256           # batches per staging/dump group
BANKS_PER_GROUP = 4
N_BANKS = B // BANK_B   # 32
N_GROUPS = 8


def split_multiwait_insts(nc):
    """walrus in this env allows only one sem wait per instruction; the tile
    tail drain carries several. Hoist extras onto preceding single-wait NoOps."""
    for func in nc.m.functions:
        for blk in func.blocks:
            insts = list(blk.instructions)
            changed = False
            new_list = []
            for inst in insts:
                si = inst.sync_info
                if si is not None and len(si.on_wait) > 1:
                    waits = list(si.on_wait)
                    for k, w in enumerate(waits[1:]):
                        new_list.append(
                            mybir.InstNoOp(
                                name=f"{inst.name}-wsplit{k}",
                                engine=inst.engine,
                                sync_info=mybir.SyncInfo(on_wait=[w], on_update=[]),
                                bass_nofuse=True,
                            )
                        )
                    inst.sync_info = mybir.SyncInfo(
                        on_wait=[waits[0]], on_update=list(si.on_update)
                    )
                    changed = True
                new_list.append(inst)
            if changed:
                blk.instructions = new_list


def host_prep(xc):
    """[2048, 32, 64] f32 -> [128, 16, 2048] fp16.

    Row 64*par + d, col (t, p, f): batch 128*t + 64*par + p."""
    xv = xc.reshape(N_LOADS, 2, 64, F, D)      # [t, par, p, f, d]
    xq = xv.transpose(1, 4, 0, 2, 3)           # [par, d, t, p, f]
    return np.ascontiguousarray(
        xq.reshape(128, N_LOADS, 2048).astype(np.float16)
    )


def build_program():
    nc = bass.Bass()
    xq = nc.declare_dram_parameter("xq", [128, N_LOADS, 2048], FP16,
                                   isOutput=False)
    dump = nc.declare_dram_parameter(
        "dump", [N_GROUPS, 128, GROUP_B * 8], FP16, isOutput=True
    )

    with tile.TileContext(nc) as tc:
        with (
            tc.tile_pool(name="xin", bufs=NB) as xpool,
            tc.tile_pool(name="stage", bufs=2) as spool,
            tc.tile_pool(name="psum_g", bufs=6, space="PSUM") as psumG,
        ):
            # persistent X buffers; zero cells written once (vector +
            # gpsimd in parallel, interleaved with the first loads so
            # buffer 0 is ready as early as possible)
            Xs = [xpool.tile([128, 2, 64, F], FP16, name=f"X{b}")
                  for b in range(NB)]

            def load(t):
                X = Xs[t % NB]
                nc.sync.dma_start(X[0:64, 0], xq[0:64, t, :])
                nc.sync.dma_start(X[64:128, 1], xq[64:128, t, :])

            for t0 in range(NB):
                nc.vector.memset(Xs[t0][64:128, 0], 0.0)
                nc.gpsimd.memset(Xs[t0][0:64, 1], 0.0)
                load(t0)
            for grp in range(N_GROUPS):
                S = spool.tile([128, BANKS_PER_GROUP, 16, F], FP16)
                s_copies = []
                for g4 in range(BANKS_PER_GROUP):
                    bank = grp * BANKS_PER_GROUP + g4
                    t, p0 = bank // 2, (bank % 2) * 32
                    X = Xs[t % NB]
                    pG = psumG.tile([128, 16, F], FP32)  # 1 bank, 16 slots
                    for i in range(BANK_B):
                        sl, j = i // 4, i % 4
                        r, p = i % 2, p0 + i // 2
                        cell = X[:, r, p, :]             # [128, 32]
                        nc.tensor.matmul(
                            pG[32 * j : 32 * j + 32, sl], lhsT=cell, rhs=cell,
                            start=True, stop=True, tile_position=(0, 32 * j),
                        )
                    # prefetch into the buffer this load just freed; issued
                    # after the load's last matmul so the WAR dep orders the
                    # DMA behind those reads, not ahead of them
                    if bank % 2 == 1 and t + NB < N_LOADS:
                        load(t + NB)
                    for half in range(2):
                        src = pG[:, 8 * half : 8 * half + 8, :]
                        dst = S[:, g4, 8 * half : 8 * half + 8, :]
                        if half == 0:
                            cp = nc.vector.tensor_copy(dst, src)
                        else:
                            cp = nc.scalar.copy(dst, src)
                        s_copies.append(cp.ins)
                g = nc.sync.dma_start(dump[grp], S[:])
                for cp_inst in s_copies:
                    add_dep_helper(g.ins, cp_inst, sync=True)

    split_multiwait_insts(nc)
    return nc


_CACHED = None


def _get_program():
    global _CACHED
    if _CACHED is None:
        _CACHED = build_program()
    return _CACHED


_TRIL_ROWS, _TRIL_COLS = np.tril_indices(F, k=-1)


def _batch_map():
    """batch index for each (grp, j, g4, sl) dump coordinate.

    dump[grp, 128p, 2048]: partition p = 32*j + f; cols (g4, sl, c).
    Bank cell i = 4*sl + j held batch 128*t + 64*(i%2) + p0 + i//2."""
    grp, j, g4, sl = np.meshgrid(
        np.arange(N_GROUPS), np.arange(4), np.arange(BANKS_PER_GROUP),
        np.arange(16), indexing="ij",
    )
    bank = grp * BANKS_PER_GROUP + g4
    t, p0 = bank // 2, (bank % 2) * 32
    i = 4 * sl + j
    return (LOAD_B * t + 64 * (i % 2) + p0 + i // 2).ravel()


_BATCH_MAP = _batch_map()


def _unpack_dump(d):
    """[8, 128, 2048] fp16 dump -> [2048, 496] f32 triangle rows."""
    d6 = d.reshape(N_GROUPS, 4, F, BANKS_PER_GROUP, 16, F)  # grp,j,f,g4,sl,c
    d6 = d6.transpose(0, 1, 3, 4, 2, 5)                     # grp,j,g4,sl,f,c
    G = np.empty((B, F, F), dtype=np.float32)
    G[_BATCH_MAP] = d6.reshape(-1, F, F).astype(np.float32)
    return G[:, _TRIL_ROWS, _TRIL_COLS]


def kernel(**inputs) -> np.ndarray:
    x = np.asarray(inputs["x"], dtype=np.float32)
    assert x.shape == (B_FULL, F, D), x.shape
    nc = _get_program()
    in_maps = [host_prep(x[i * B : (i + 1) * B]) for i in range(N_CORES)]
    res = run_bass_kernel_spmd(
        nc, [{"xq": m} for m in in_maps], list(range(N_CORES))
    )
    return np.concatenate(
        [_unpack_dump(res.results[i]["dump"]) for i in range(N_CORES)], axis=0
    ).astype(np.float32)
